# revision 18
# baseline (speedup 1.0000x reference)
"""Trainium2 Bass kernel for nn_Downsampler: depthwise 4x4 conv, stride 4,
VALID padding, one shared (runtime) 4x4 kernel across all channels.

  x: (16, 8, 1024, 1024) f32, kernel: (4, 4) f32 -> out: (16, 8, 256, 256) f32

Sharding: pure data parallel over batch N=16 -> 2 batches per core on 8 cores.

Math: out[o, j] = sum_{di,dj} k[di,dj] * x[4o+di, 4j+dj], rows flattened over
(n, c, h) since every image row has W=1024 and slabs never straddle an (n, c)
boundary (1024 rows per image, slab = 512 rows).

The whole conv runs on the TensorEngine: per slab of 512 input rows (SBUF
tile [128, 4096], partition p, quarter d -> row 512*s + 128*d + p), the
output rows 32*d + m (m = p//4) are

    psum[m, 256*d + j] = sum_dj sum_p selg_dj[p, m] * xt[p, (d, 4j+dj)]

with selg_dj[p, m] = kernel[p%4, dj] * (p//4 == m), i.e. 4 row-quarters x
4 dj-phases = 16 accumulating matmuls, each N=256 with a stride-4 rhs view.
This is exact for an ARBITRARY 4x4 kernel (no separability assumption) and
costs the same PE time as a rank-1 pass because each phase streams 256
columns instead of 1024. float32r runs the PE at 1 col/cycle (plain fp32
matmul lowers to 2 half-speed passes = 4x slower); precision loss is well
inside the 2e-2 gate.

fp32r occupies 2 PE array columns per weight column, so a 32-output matmul
is only placeable at col-group offsets 0 or 64 (s3d3_mm_valid_dst_partition)
-- quarters therefore land side by side in PSUM *columns* of one [32, 1024]
tile (2 banks) at partition base 0, not stacked by tile_position.

Vector/GpSimd do nothing; ScalarE only evicts PSUM -> SBUF (DMA cannot read
PSUM) and issues the output DMA on the ACT HWDGE ring, keeping the SP ring
a pure input stream. Per slab the output is y[128s : 128s+128, :] with row
32d+m taken from ot[m, 256d:256d+256].

Each quarter is its own accumulation group (start on dj=0, stop on dj=3).
A group-start clears has_written bits bank-wide, but the PE executes
matmuls in strict program order, so the earlier quarter sharing the bank is
complete before the clear -- the bits only gate accumulate-vs-overwrite,
and nothing accumulates onto a finished quarter afterwards.

A dummy keep-warm matmul per slab fills the PE's inter-slab wait gaps so
the HAM clock gate stays at K=8/8 (2.4 GHz).
"""

import json
from contextlib import ExitStack

import numpy as np

import concourse.bass as bass
import concourse.mybir as mybir
from concourse.tile import TileContext
from concourse.bass_utils import run_bass_kernel_spmd

N, C, H, W = 16, 8, 1024, 1024
F = 4
N_CORES = 8
R = (N // N_CORES) * C * H  # input rows per core (16384)
WO = W // F  # output row length (256)


def _split_excess_waits(bir_bytes: bytes, max_waits: int = 1) -> bytes:
    """The public neuronxcc walrus supports at most ONE sync wait per
    instruction; hoist excess waits onto NoOps inserted just before."""
    m = json.loads(bir_bytes)

    def fix(blocks):
        for bb in blocks:
            out = []
            for ins in bb.get("instructions", []):
                si = ins.get("sync_info")
                waits = (si or {}).get("on_wait") or []
                if len(waits) > max_waits:
                    extra = waits[:-max_waits]
                    si["on_wait"] = waits[-max_waits:]
                    for i in range(0, len(extra), max_waits):
                        out.append(
                            {
                                "debug": ins.get("debug", 0),
                                "engine": ins["engine"],
                                "ins": [],
                                "outs": [],
                                "name": f"{ins['name']}-ws{i}",
                                "opcode": "NoOp",
                                "sync_info": {
                                    "on_update": [],
                                    "on_wait": extra[i : i + max_waits],
                                },
                            }
                        )
                out.append(ins)
            bb["instructions"] = out
            fix(bb.get("blocks", []))

    for f in m["functions"]:
        fix(f["blocks"])
    return json.dumps(m).encode()


def _make_selg(kernel: np.ndarray) -> np.ndarray:
    """PE stationary weights [128, 4*32]: selg[p, 32*dj + m] =
    kernel[p%4, dj] * (p//4 == m)."""
    kernel = np.asarray(kernel, dtype=np.float32)
    assert kernel.shape == (F, F)
    selg = np.zeros((128, 128), dtype=np.float32)
    p = np.arange(128)
    for dj in range(F):
        selg[p, 32 * dj + p // F] = kernel[p % F, dj]
    return selg


def _build_nc(rows: int, xt_bufs: int = 8, psum_bufs: int = 6, o_bufs: int = 6) -> bass.Bass:
    assert rows % 1024 == 0
    n_slabs = rows // 512

    f32 = mybir.dt.float32
    f32r = mybir.dt.float32r

    nc = bass.Bass("TRN2", target_bir_lowering=False, debug=False)
    x = nc.dram_tensor("x", [rows, W], f32r, kind="ExternalInput")
    selg = nc.dram_tensor("selg", [128, 4 * 32], f32r, kind="ExternalInput")
    y = nc.dram_tensor("y", [rows // F, WO], f32, kind="ExternalOutput")

    with TileContext(nc) as tc:
        with ExitStack() as ctx:
            const_pool = ctx.enter_context(tc.tile_pool(name="const_pool", bufs=1))
            selgt = const_pool.tile([128, 4 * 32], f32r)
            # const load rides the ACT ring so the SP ring is input-only
            nc.scalar.dma_start(selgt[:], selg.ap())

            x_pool = ctx.enter_context(tc.tile_pool(name="x_pool", bufs=xt_bufs))
            ps_pool = ctx.enter_context(
                tc.tile_pool(name="ps_pool", bufs=psum_bufs, space="PSUM")
            )
            o_pool = ctx.enter_context(tc.tile_pool(name="o_pool", bufs=o_bufs))

            # HAM warm-up: without this the PE never sees a fully-busy
            # 4096-cycle window (real work is ~75%-duty bursts) and stays
            # clock-gated at K=4/8 (1.2 GHz) for the WHOLE kernel. A one-off
            # ~4us burst of back-to-back dummy matmuls -- executed while the
            # PE would be idle waiting for the first input tile anyway --
            # flips it to 8/8, and the steady ~2.65us work cadence keeps it
            # there (re-throttle needs a fully-idle window).
            wp_pool = ctx.enter_context(
                tc.tile_pool(name="wp_pool", bufs=1, space="PSUM")
            )
            warm_pt = wp_pool.tile([32, 128], f32)
            for _ in range(36):
                nc.tensor.matmul(
                    warm_pt[:], selgt[:, 0:32], selgt[:], start=True, stop=True
                )

            for s in range(n_slabs):
                # per-HALF-SLAB input DMAs (256 rows, 1 MiB each): each
                # half's 4 matmuls depend only on their own half, so the PE
                # gets work every ~3us (HAM stays at K=8/8, no keep-warm
                # needed) and the tail drains at 1 MiB grain
                xt = x_pool.tile([128, 4 * W], f32r, name="xt")
                # xv[p, h, d2, j, q] = xt[p, (2h+d2)*1024 + 4j + q]; the
                # matmul rhs for (h, dj) spans both quarters of the half --
                # the quarter index rides the FREE axis (N=512, one full
                # PSUM bank), halving the matmul count vs per-quarter MMs
                xv = xt[:].rearrange("p (h d2 j q) -> p h d2 j q", h=2, d2=2, q=F)

                for h in range(2):
                    r0 = s * 512 + h * 256
                    nc.sync.dma_start(
                        xt[:, 2 * h * W : 2 * (h + 1) * W].rearrange(
                            "p (d w) -> p d w", d=2
                        ),
                        x.ap()[r0 : r0 + 256, :].rearrange(
                            "(d p) w -> p d w", p=128
                        ),
                    )
                    # per-half PSUM tile (exactly one bank) so h0's
                    # eviction never false-serializes against h1's matmuls
                    pt = ps_pool.tile([32, 2 * WO], f32, name="pt")
                    for dj in range(4):
                        nc.tensor.matmul(
                            pt[:],
                            selgt[:, 32 * dj : 32 * dj + 32],
                            xv[:, h, :, :, dj],
                            start=(dj == 0),
                            stop=(dj == 3),
                        )
                    # evict PSUM -> SBUF per half (DMA cannot read PSUM),
                    # alternating engines so the two halves' tails run in
                    # parallel, then one output DMA per half:
                    # y row 128s+64h+32d2+m <- ot[m, 256*d2+j]
                    ot = o_pool.tile([32, 2 * WO], f32, name="ot")
                    dst = y.ap()[
                        128 * s + 64 * h : 128 * s + 64 * h + 64, :
                    ].rearrange("(d m) j -> m d j", d=2)
                    if h == 0:
                        nc.scalar.copy(ot[:], pt[:])
                        nc.scalar.dma_start(
                            dst, ot[:].rearrange("m (d j) -> m d j", d=2)
                        )
                    else:
                        # NOTE: output DMAs must NOT ride the sync ring --
                        # their wait would block later INPUT DMAs (in-order
                        # queue) and stall the whole stream
                        nc.vector.tensor_copy(ot[:], pt[:])
                        nc.scalar.dma_start(
                            dst, ot[:].rearrange("m (d j) -> m d j", d=2)
                        )

    # walrus 1-wait-per-instruction workaround, applied at serialization time
    orig = nc.to_json_bytes
    nc.to_json_bytes = lambda: _split_excess_waits(orig())
    return nc


_NC_CACHE: dict[int, bass.Bass] = {}


def _get_nc(rows: int = R) -> bass.Bass:
    if rows not in _NC_CACHE:
        _NC_CACHE[rows] = _build_nc(rows)
    return _NC_CACHE[rows]


def run_spmd(x: np.ndarray, kern: np.ndarray, **spmd_kwargs):
    """Shard, run on 8 cores, gather. Returns (output, BassKernelResults)."""
    assert x.shape == (N, C, H, W) and kern.shape == (F, F)
    x = np.ascontiguousarray(x, dtype=np.float32)
    selg = _make_selg(kern)
    nb = N // N_CORES
    in_maps = [
        {"x": x[i * nb : (i + 1) * nb].reshape(R, W), "selg": selg}
        for i in range(N_CORES)
    ]
    nc = _get_nc()
    res = run_bass_kernel_spmd(
        nc, in_maps, core_ids=list(range(N_CORES)), **spmd_kwargs
    )
    out = np.concatenate(
        [res.results[i]["y"].reshape(nb, C, H // F, WO) for i in range(N_CORES)],
        axis=0,
    )
    return out, res


def kernel(x: np.ndarray, kernel: np.ndarray) -> np.ndarray:
    out, _ = run_spmd(x, kernel)
    return out


# revision 20
# speedup vs baseline: 1.0049x; 1.0049x over previous
"""Trainium2 Bass kernel for nn_Downsampler: depthwise 4x4 conv, stride 4,
VALID padding, one shared (runtime) 4x4 kernel across all channels.

  x: (16, 8, 1024, 1024) f32, kernel: (4, 4) f32 -> out: (16, 8, 256, 256) f32

Sharding: pure data parallel over batch N=16 -> 2 batches per core on 8 cores.

Math: out[o, j] = sum_{di,dj} k[di,dj] * x[4o+di, 4j+dj], rows flattened over
(n, c, h) since every image row has W=1024 and slabs never straddle an (n, c)
boundary (1024 rows per image, slab = 512 rows).

The whole conv runs on the TensorEngine: per slab of 512 input rows (SBUF
tile [128, 4096], partition p, quarter d -> row 512*s + 128*d + p), the
output rows 32*d + m (m = p//4) are

    psum[m, 256*d + j] = sum_dj sum_p selg_dj[p, m] * xt[p, (d, 4j+dj)]

with selg_dj[p, m] = kernel[p%4, dj] * (p//4 == m), i.e. 4 row-quarters x
4 dj-phases = 16 accumulating matmuls, each N=256 with a stride-4 rhs view.
This is exact for an ARBITRARY 4x4 kernel (no separability assumption) and
costs the same PE time as a rank-1 pass because each phase streams 256
columns instead of 1024. float32r runs the PE at 1 col/cycle (plain fp32
matmul lowers to 2 half-speed passes = 4x slower); precision loss is well
inside the 2e-2 gate.

fp32r occupies 2 PE array columns per weight column, so a 32-output matmul
is only placeable at col-group offsets 0 or 64 (s3d3_mm_valid_dst_partition)
-- quarters therefore land side by side in PSUM *columns* of one [32, 1024]
tile (2 banks) at partition base 0, not stacked by tile_position.

Vector/GpSimd do nothing; ScalarE only evicts PSUM -> SBUF (DMA cannot read
PSUM) and issues the output DMA on the ACT HWDGE ring, keeping the SP ring
a pure input stream. Per slab the output is y[128s : 128s+128, :] with row
32d+m taken from ot[m, 256d:256d+256].

Each quarter is its own accumulation group (start on dj=0, stop on dj=3).
A group-start clears has_written bits bank-wide, but the PE executes
matmuls in strict program order, so the earlier quarter sharing the bank is
complete before the clear -- the bits only gate accumulate-vs-overwrite,
and nothing accumulates onto a finished quarter afterwards.

A dummy keep-warm matmul per slab fills the PE's inter-slab wait gaps so
the HAM clock gate stays at K=8/8 (2.4 GHz).
"""

import json
from contextlib import ExitStack

import numpy as np

import concourse.bass as bass
import concourse.mybir as mybir
from concourse.tile import TileContext
from concourse.bass_utils import run_bass_kernel_spmd

N, C, H, W = 16, 8, 1024, 1024
F = 4
N_CORES = 8
R = (N // N_CORES) * C * H  # input rows per core (16384)
WO = W // F  # output row length (256)


def _split_excess_waits(bir_bytes: bytes, max_waits: int = 1) -> bytes:
    """The public neuronxcc walrus supports at most ONE sync wait per
    instruction; hoist excess waits onto NoOps inserted just before."""
    m = json.loads(bir_bytes)

    def fix(blocks):
        for bb in blocks:
            out = []
            for ins in bb.get("instructions", []):
                si = ins.get("sync_info")
                waits = (si or {}).get("on_wait") or []
                if len(waits) > max_waits:
                    extra = waits[:-max_waits]
                    si["on_wait"] = waits[-max_waits:]
                    for i in range(0, len(extra), max_waits):
                        out.append(
                            {
                                "debug": ins.get("debug", 0),
                                "engine": ins["engine"],
                                "ins": [],
                                "outs": [],
                                "name": f"{ins['name']}-ws{i}",
                                "opcode": "NoOp",
                                "sync_info": {
                                    "on_update": [],
                                    "on_wait": extra[i : i + max_waits],
                                },
                            }
                        )
                out.append(ins)
            bb["instructions"] = out
            fix(bb.get("blocks", []))

    for f in m["functions"]:
        fix(f["blocks"])
    return json.dumps(m).encode()


def _make_selg(kernel: np.ndarray) -> np.ndarray:
    """PE stationary weights [128, 4*32]: selg[p, 32*dj + m] =
    kernel[p%4, dj] * (p//4 == m)."""
    kernel = np.asarray(kernel, dtype=np.float32)
    assert kernel.shape == (F, F)
    selg = np.zeros((128, 128), dtype=np.float32)
    p = np.arange(128)
    for dj in range(F):
        selg[p, 32 * dj + p // F] = kernel[p % F, dj]
    return selg


def _build_nc(rows: int, xt_bufs: int = 12, psum_bufs: int = 6, o_bufs: int = 4) -> bass.Bass:
    assert rows % 1024 == 0
    n_slabs = rows // 512

    f32 = mybir.dt.float32
    f32r = mybir.dt.float32r

    nc = bass.Bass("TRN2", target_bir_lowering=False, debug=False)
    x = nc.dram_tensor("x", [rows, W], f32r, kind="ExternalInput")
    selg = nc.dram_tensor("selg", [128, 4 * 32], f32r, kind="ExternalInput")
    y = nc.dram_tensor("y", [rows // F, WO], f32, kind="ExternalOutput")

    with TileContext(nc) as tc:
        with ExitStack() as ctx:
            const_pool = ctx.enter_context(tc.tile_pool(name="const_pool", bufs=1))
            selgt = const_pool.tile([128, 4 * 32], f32r)
            # const load rides the ACT ring so the SP ring is input-only
            nc.scalar.dma_start(selgt[:], selg.ap())

            x_pool = ctx.enter_context(tc.tile_pool(name="x_pool", bufs=xt_bufs))
            ps_pool = ctx.enter_context(
                tc.tile_pool(name="ps_pool", bufs=psum_bufs, space="PSUM")
            )
            o_pool = ctx.enter_context(tc.tile_pool(name="o_pool", bufs=o_bufs))

            # (No HAM warm-up: measured on HW, the stride-4 rhs matmuls run
            # ~460ns at K=8/8 and K=4/8 alike -- the strided read caps the
            # stream rate at ~2.15 cyc/col, so PE clock state is irrelevant.)

            for s in range(n_slabs):
                # per-HALF-SLAB input DMAs (256 rows, 1 MiB each): each
                # half's 4 matmuls depend only on their own half, so the PE
                # gets work every ~3us (HAM stays at K=8/8, no keep-warm
                # needed) and the tail drains at 1 MiB grain
                xt = x_pool.tile([128, 4 * W], f32r, name="xt")
                # xv[p, h, d2, j, q] = xt[p, (2h+d2)*1024 + 4j + q]; the
                # matmul rhs for (h, dj) spans both quarters of the half --
                # the quarter index rides the FREE axis (N=512, one full
                # PSUM bank), halving the matmul count vs per-quarter MMs
                xv = xt[:].rearrange("p (h d2 j q) -> p h d2 j q", h=2, d2=2, q=F)

                for h in range(2):
                    r0 = s * 512 + h * 256
                    nc.sync.dma_start(
                        xt[:, 2 * h * W : 2 * (h + 1) * W].rearrange(
                            "p (d w) -> p d w", d=2
                        ),
                        x.ap()[r0 : r0 + 256, :].rearrange(
                            "(d p) w -> p d w", p=128
                        ),
                    )
                    # per-half PSUM tile (exactly one bank) so h0's
                    # eviction never false-serializes against h1's matmuls
                    pt = ps_pool.tile([32, 2 * WO], f32, name="pt")
                    for dj in range(4):
                        nc.tensor.matmul(
                            pt[:],
                            selgt[:, 32 * dj : 32 * dj + 32],
                            xv[:, h, :, :, dj],
                            start=(dj == 0),
                            stop=(dj == 3),
                        )
                    # evict PSUM -> SBUF per half (DMA cannot read PSUM),
                    # alternating engines so the two halves' tails run in
                    # parallel, then one output DMA per half:
                    # y row 128s+64h+32d2+m <- ot[m, 256*d2+j]
                    ot = o_pool.tile([32, 2 * WO], f32, name="ot")
                    dst = y.ap()[
                        128 * s + 64 * h : 128 * s + 64 * h + 64, :
                    ].rearrange("(d m) j -> m d j", d=2)
                    if h == 0:
                        nc.scalar.copy(ot[:], pt[:])
                        nc.scalar.dma_start(
                            dst, ot[:].rearrange("m (d j) -> m d j", d=2)
                        )
                    else:
                        # NOTE: output DMAs must NOT ride the sync ring --
                        # their wait would block later INPUT DMAs (in-order
                        # queue) and stall the whole stream
                        nc.vector.tensor_copy(ot[:], pt[:])
                        nc.scalar.dma_start(
                            dst, ot[:].rearrange("m (d j) -> m d j", d=2)
                        )

    # walrus 1-wait-per-instruction workaround, applied at serialization time
    orig = nc.to_json_bytes
    nc.to_json_bytes = lambda: _split_excess_waits(orig())
    return nc


_NC_CACHE: dict[int, bass.Bass] = {}


def _get_nc(rows: int = R) -> bass.Bass:
    if rows not in _NC_CACHE:
        _NC_CACHE[rows] = _build_nc(rows)
    return _NC_CACHE[rows]


def run_spmd(x: np.ndarray, kern: np.ndarray, **spmd_kwargs):
    """Shard, run on 8 cores, gather. Returns (output, BassKernelResults)."""
    assert x.shape == (N, C, H, W) and kern.shape == (F, F)
    x = np.ascontiguousarray(x, dtype=np.float32)
    selg = _make_selg(kern)
    nb = N // N_CORES
    in_maps = [
        {"x": x[i * nb : (i + 1) * nb].reshape(R, W), "selg": selg}
        for i in range(N_CORES)
    ]
    nc = _get_nc()
    res = run_bass_kernel_spmd(
        nc, in_maps, core_ids=list(range(N_CORES)), **spmd_kwargs
    )
    out = np.concatenate(
        [res.results[i]["y"].reshape(nb, C, H // F, WO) for i in range(N_CORES)],
        axis=0,
    )
    return out, res


def kernel(x: np.ndarray, kernel: np.ndarray) -> np.ndarray:
    out, _ = run_spmd(x, kernel)
    return out
